# revision 25
# baseline (speedup 1.0000x reference)
"""Sobel filter Trainium2 Bass kernel.

Problem: img [32, 3, 512, 512] f32, kx/ky [1, 3, 3, 3] f32 (same 3x3 kernel
broadcast over the 3 input channels in the reference, but we honor arbitrary
values). Output [32, 1, 512, 512] f32:
    Gx = valid_conv3x3(img, kx), Gy = valid_conv3x3(img, ky)  -> [N,1,510,510]
    out = sqrt(Gx^2 + Gy^2) edge-padded by 1 back to [N,1,512,512]

Strategy (pure data parallel over 8 NeuronCores, 4 images per core):
  The 2D conv runs on the TensorEngine as sums of banded-Toeplitz matmuls.
  Partition dim = image rows (y). For each (channel c, x-shift dx) the 3-tap
  y-convolution is a banded [K=128, M=126] stationary matrix
  A[k, m] = w[c, k-m, dx]; the moving operand is the x-shifted image rows
  img[c, y0:y0+128, dx:dx+510]. Summing over (c, dx) for each of Gx/Gy is
  PSUM accumulation over 9 matmuls -> [126, 510] valid conv rows per PSUM
  tile. 4 row-tiles of 126 cover rows 0..503; the remaining 6 valid rows of
  ALL 4 images are computed by one extra "mini" tile with a block-diagonal
  [32, 24] stationary (4 blocks of [8 in-rows, 6 out-rows]).

  Matmul operands are float32r (full-rate fp32 matmul mode; plain float32
  streams at 1/4 rate). Loads use 128-partition DMAs (104-partition DMAs
  measured at 159 GB/s vs 286 GB/s for 128). Magnitude epilogue: squares on
  ScalarE (PSUM->SBUF), add on VectorE, sqrt on ScalarE; column edge padding
  in-SBUF, row edge padding via small extra stores.

The banded stationary matrices (built from kx/ky on host) are passed as
replicated input tensors.
"""

import os

import numpy as np

N_CORES = 8
N_FULL = 32          # full batch
N_PER_CORE = N_FULL // N_CORES
H = W = 512
TILE_K = 128         # input rows per full row-tile
TILE_M = 126         # valid output rows per full row-tile
N_TILES = 4          # 4 * 126 = 504 valid rows; remaining 6 via mini tile
NW = 510             # valid output columns
MINI_K = 8 * N_PER_CORE   # 4 images x 8 input rows
MINI_M = 6 * N_PER_CORE   # 4 images x 6 output rows

_CACHE: dict = {}
LAST_RESULTS = None  # BassKernelResults of the most recent run (for test.py)


def _build_stationaries(kx: np.ndarray, ky: np.ndarray):
    """Returns (stat [TILE_K, 18, TILE_M], stat_mini [MINI_K, 18, MINI_M]).
    Slice i=(g,c,dx) of stat is the banded matrix A[k, m] = kG[c, k-m, dx]
    for k-m in {0,1,2}; stat_mini is block-diagonal per image."""
    ks = (np.asarray(kx, np.float32), np.asarray(ky, np.float32))
    stat = np.zeros((18, TILE_K, TILE_M), np.float32)
    mini = np.zeros((18, MINI_K, MINI_M), np.float32)
    m = np.arange(TILE_M)
    mm = np.arange(6)
    i = 0
    for g in range(2):
        for c in range(3):
            for dx in range(3):
                for dy in range(3):
                    stat[i, m + dy, m] = ks[g][0, c, dy, dx]
                    for j in range(N_PER_CORE):
                        mini[i, j * 8 + mm + dy, j * 6 + mm] = ks[g][0, c, dy, dx]
                i += 1
    return (
        np.ascontiguousarray(stat.transpose(1, 0, 2)),
        np.ascontiguousarray(mini.transpose(1, 0, 2)),
    )


def _epilogue(nc, work_pool, psx, psy, rows, f32):
    """sqrt(psx^2 + psy^2) -> [rows, 512] SBUF tile with edge cols."""
    s = work_pool.tile([rows, W], f32, tag="s", name="s")
    s2 = work_pool.tile([rows, NW], f32, tag="s2", name="s2")
    nc.scalar.square(s[:, 1 : 1 + NW], psx)
    nc.scalar.square(s2, psy)
    nc.vector.tensor_add(s[:, 1 : 1 + NW], s[:, 1 : 1 + NW], s2)
    nc.vector.tensor_copy(s[:, 0:1], s[:, 1:2])
    nc.vector.tensor_copy(s[:, W - 1 : W], s[:, W - 2 : W - 1])
    mag = work_pool.tile([rows, W], f32, tag="mag", name="mag")
    nc.scalar.sqrt(mag, s)
    return mag


def _sobel_body(tc, out, img, stat_dram, stat_mini_dram):
    import concourse.mybir as mybir

    nc = tc.nc
    f32 = mybir.dt.float32
    mm_dt = mybir.dt.float32r

    img_yx = img.rearrange("n c y x -> n y c x")

    with (
        tc.tile_pool(name="const", bufs=1) as const_pool,
        tc.tile_pool(name="imgs", bufs=3) as img_pool,
        tc.tile_pool(name="work", bufs=3) as work_pool,
        tc.tile_pool(name="psum", bufs=2, space="PSUM") as psum_pool,
    ):
        # Load order is tuned so the PE can start early: the tiny mini-tile
        # inputs go first on the sync ring, so the mini matmuls do useful
        # work (and ramp the PE clock) while the big stat matrix and first
        # image tiles stream in behind them.
        stat_mini_sb = const_pool.tile([MINI_K, 18, MINI_M], mm_dt)
        nc.sync.dma_start(out=stat_mini_sb, in_=stat_mini_dram)
        # per-channel 32-partition DMAs (narrower DMAs steal
        # disproportionate SDMA-engine time)
        mit = img_pool.tile([MINI_K, 3, W], mm_dt, tag="mit", bufs=1)
        for c in range(3):
            nc.sync.dma_start(out=mit[:, c, :], in_=img_yx[:, H - 8 : H, c])
        # stat piece-pairs in MM order: Gx pairs (0-4) on the sync ring ahead
        # of the image loads; Gy pairs (5-8) on the scalar ring, whose
        # triggers sit behind the ~2.7us ACT table load.
        stat_sb = const_pool.tile([TILE_K, 18, TILE_M], mm_dt)
        for j in range(5):
            nc.sync.dma_start(
                out=stat_sb[:, 2 * j : 2 * j + 2], in_=stat_dram[:, 2 * j : 2 * j + 2]
            )
        for j in range(5, 9):
            nc.scalar.dma_start(
                out=stat_sb[:, 2 * j : 2 * j + 2], in_=stat_dram[:, 2 * j : 2 * j + 2]
            )

        def big_tile(n, t):
            y0 = t * TILE_M
            # per-channel loads -> finer-grained MM/DMA pipelining. All loads
            # on the sync HWDGE ring, all stores on the scalar ring: measured
            # 287 GB/s vs 215 GB/s with loads+stores sharing a ring.
            its = []
            for c in range(3):
                itc = img_pool.tile(
                    [TILE_K, W], mm_dt, tag=f"it{c}", name=f"it{c}", bufs=4
                )
                nc.sync.dma_start(out=itc, in_=img_yx[n, y0 : y0 + TILE_K, c])
                its.append(itc)

            psx = psum_pool.tile([TILE_M, NW], f32, tag="psx", name="psx")
            psy = psum_pool.tile([TILE_M, NW], f32, tag="psy", name="psy")
            for g, ps in ((0, psx), (1, psy)):
                mmi = 0
                for c in range(3):
                    for dx in range(3):
                        i = (g * 3 + c) * 3 + dx
                        nc.tensor.matmul(
                            ps,
                            stat_sb[:, i, :],
                            its[c][:, dx : dx + NW],
                            start=(mmi == 0),
                            stop=(mmi == 8),
                        )
                        mmi += 1

            mag = _epilogue(nc, work_pool, psx, psy, TILE_M, f32)
            nc.scalar.dma_start(out=out[n, 1 + y0 : 1 + y0 + TILE_M, :], in_=mag)
            if t == 0:
                nc.scalar.dma_start(out=out[n, 0:1, :], in_=mag[0:1, :])

        def mini_tile():
            # last 6 valid rows (y' = 504..509) of all 4 images at once,
            # via a block-diagonal stationary
            mpsx = psum_pool.tile([MINI_M, NW], f32, tag="mpsx", bufs=1, name="mpsx")
            mpsy = psum_pool.tile([MINI_M, NW], f32, tag="mpsy", bufs=1, name="mpsy")
            for g, ps in ((0, mpsx), (1, mpsy)):
                mmi = 0
                for c in range(3):
                    for dx in range(3):
                        i = (g * 3 + c) * 3 + dx
                        nc.tensor.matmul(
                            ps,
                            stat_mini_sb[:, i, :],
                            mit[:, c, dx : dx + NW],
                            start=(mmi == 0),
                            stop=(mmi == 8),
                        )
                        mmi += 1
            mmag = _epilogue(nc, work_pool, mpsx, mpsy, MINI_M, f32)
            for n in range(N_PER_CORE):
                nc.scalar.dma_start(
                    out=out[n, H - 7 : H - 1, :], in_=mmag[n * 6 : n * 6 + 6]
                )
                nc.scalar.dma_start(
                    out=out[n, H - 1 : H, :], in_=mmag[n * 6 + 5 : n * 6 + 6]
                )

        mini_tile()
        for n in range(N_PER_CORE):
            for t in range(N_TILES):
                big_tile(n, t)


def _build_program():
    import concourse.bacc as bacc
    import concourse.mybir as mybir
    import concourse.tile as tile

    nc = bacc.Bacc(
        "TRN2",
        target_bir_lowering=False,
        debug=False,
        num_devices=N_CORES,
    )
    img = nc.dram_tensor(
        "img", [N_PER_CORE, 3, H, W], mybir.dt.float32r, kind="ExternalInput"
    ).ap()
    stat = nc.dram_tensor(
        "stat", [TILE_K, 18, TILE_M], mybir.dt.float32r, kind="ExternalInput"
    ).ap()
    stat_mini = nc.dram_tensor(
        "stat_mini", [MINI_K, 18, MINI_M], mybir.dt.float32r, kind="ExternalInput"
    ).ap()
    out = nc.dram_tensor(
        "out", [N_PER_CORE, H, W], mybir.dt.float32, kind="ExternalOutput"
    ).ap()

    with tile.TileContext(nc) as tc:
        _sobel_body(tc, out, img, stat, stat_mini)
    nc.compile()
    return nc


def kernel(img: np.ndarray, kx: np.ndarray, ky: np.ndarray) -> np.ndarray:
    global LAST_RESULTS
    from concourse.bass_utils import run_bass_kernel_spmd

    img = np.ascontiguousarray(np.asarray(img, dtype=np.float32))
    assert img.shape == (N_FULL, 3, H, W), img.shape
    stat, stat_mini = _build_stationaries(kx, ky)

    if "nc" not in _CACHE:
        _CACHE["nc"] = _build_program()
    nc = _CACHE["nc"]

    in_maps = [
        {
            "img": img[c * N_PER_CORE : (c + 1) * N_PER_CORE],
            "stat": stat,
            "stat_mini": stat_mini,
        }
        for c in range(N_CORES)
    ]
    trace = os.environ.get("SOBEL_TRACE", "0") == "1"
    res = run_bass_kernel_spmd(
        nc, in_maps, core_ids=list(range(N_CORES)), trace=trace
    )
    LAST_RESULTS = res
    out = np.concatenate([res.results[c]["out"] for c in range(N_CORES)], axis=0)
    return out.reshape(N_FULL, 1, H, W)


# revision 26
# speedup vs baseline: 1.1775x; 1.1775x over previous
"""Sobel filter Trainium2 Bass kernel.

Problem: img [32, 3, 512, 512] f32, kx/ky [1, 3, 3, 3] f32 (same 3x3 kernel
broadcast over the 3 input channels in the reference, but we honor arbitrary
values). Output [32, 1, 512, 512] f32:
    Gx = valid_conv3x3(img, kx), Gy = valid_conv3x3(img, ky)  -> [N,1,510,510]
    out = sqrt(Gx^2 + Gy^2) edge-padded by 1 back to [N,1,512,512]

Strategy (pure data parallel over 8 NeuronCores, 4 images per core):
  The 2D conv runs on the TensorEngine as sums of banded-Toeplitz matmuls.
  Partition dim = image rows (y). For each (channel c, x-shift dx) the 3-tap
  y-convolution is a banded [K=128, M=126] stationary matrix
  A[k, m] = w[c, k-m, dx]; the moving operand is the x-shifted image rows
  img[c, y0:y0+128, dx:dx+510]. Summing over (c, dx) for each of Gx/Gy is
  PSUM accumulation over 9 matmuls -> [126, 510] valid conv rows per PSUM
  tile. 4 row-tiles of 126 cover rows 0..503; the remaining 6 valid rows of
  ALL 4 images are computed by one extra "mini" tile with a block-diagonal
  [32, 24] stationary (4 blocks of [8 in-rows, 6 out-rows]).

  Matmul operands are float32r (full-rate fp32 matmul mode; plain float32
  streams at 1/4 rate). Loads use 128-partition DMAs (104-partition DMAs
  measured at 159 GB/s vs 286 GB/s for 128). Magnitude epilogue: squares on
  ScalarE (PSUM->SBUF), add on VectorE, sqrt on ScalarE; column edge padding
  in-SBUF, row edge padding via small extra stores.

The banded stationary matrices (built from kx/ky on host) are passed as
replicated input tensors.
"""

import os

import numpy as np

N_CORES = 8
N_FULL = 32          # full batch
N_PER_CORE = N_FULL // N_CORES
H = W = 512
TILE_K = 128         # input rows per full row-tile
TILE_M = 126         # valid output rows per full row-tile
N_TILES = 4          # 4 * 126 = 504 valid rows; remaining 6 via mini tile
NW = 510             # valid output columns
MINI_K = 8 * N_PER_CORE   # 4 images x 8 input rows
MINI_M = 6 * N_PER_CORE   # 4 images x 6 output rows

_CACHE: dict = {}
LAST_RESULTS = None  # BassKernelResults of the most recent run (for test.py)


def _build_stationaries(kx: np.ndarray, ky: np.ndarray):
    """Returns (stat [TILE_K, 18, TILE_M], stat_mini [MINI_K, 18, MINI_M]).
    Slice i=(g,c,dx) of stat is the banded matrix A[k, m] = kG[c, k-m, dx]
    for k-m in {0,1,2}; stat_mini is block-diagonal per image."""
    ks = (np.asarray(kx, np.float32), np.asarray(ky, np.float32))
    stat = np.zeros((18, TILE_K, TILE_M), np.float32)
    mini = np.zeros((18, MINI_K, MINI_M), np.float32)
    m = np.arange(TILE_M)
    mm = np.arange(6)
    i = 0
    for g in range(2):
        for c in range(3):
            for dx in range(3):
                for dy in range(3):
                    stat[i, m + dy, m] = ks[g][0, c, dy, dx]
                    for j in range(N_PER_CORE):
                        mini[i, j * 8 + mm + dy, j * 6 + mm] = ks[g][0, c, dy, dx]
                i += 1
    return (
        np.ascontiguousarray(stat.transpose(1, 0, 2)),
        np.ascontiguousarray(mini.transpose(1, 0, 2)),
    )


def _epilogue(nc, work_pool, psx, psy, rows, f32):
    """sqrt(psx^2 + psy^2) -> [rows, 512] SBUF tile with edge cols."""
    s = work_pool.tile([rows, W], f32, tag="s", name="s")
    s2 = work_pool.tile([rows, NW], f32, tag="s2", name="s2")
    nc.scalar.square(s[:, 1 : 1 + NW], psx)
    nc.scalar.square(s2, psy)
    nc.vector.tensor_add(s[:, 1 : 1 + NW], s[:, 1 : 1 + NW], s2)
    nc.vector.tensor_copy(s[:, 0:1], s[:, 1:2])
    nc.vector.tensor_copy(s[:, W - 1 : W], s[:, W - 2 : W - 1])
    mag = work_pool.tile([rows, W], f32, tag="mag", name="mag")
    nc.scalar.sqrt(mag, s)
    return mag


def _sobel_body(tc, out, img, stat_dram, stat_mini_dram):
    import concourse.mybir as mybir

    nc = tc.nc
    f32 = mybir.dt.float32
    mm_dt = mybir.dt.float32r

    img_yx = img.rearrange("n c y x -> n y c x")

    with (
        tc.tile_pool(name="const", bufs=1) as const_pool,
        tc.tile_pool(name="imgs", bufs=3) as img_pool,
        tc.tile_pool(name="work", bufs=3) as work_pool,
        tc.tile_pool(name="psum", bufs=2, space="PSUM") as psum_pool,
    ):
        # Load order is tuned so the PE can start early: the tiny mini-tile
        # inputs go first on the sync ring, so the mini matmuls do useful
        # work (and ramp the PE clock) while the big stat matrix and first
        # image tiles stream in behind them.
        stat_mini_sb = const_pool.tile([MINI_K, 18, MINI_M], mm_dt)
        nc.sync.dma_start(out=stat_mini_sb, in_=stat_mini_dram)
        # per-channel 32-partition DMAs (narrower DMAs steal
        # disproportionate SDMA-engine time)
        mit = img_pool.tile([MINI_K, 3, W], mm_dt, tag="mit", bufs=1)
        for c in range(3):
            nc.sync.dma_start(out=mit[:, c, :], in_=img_yx[:, H - 8 : H, c])
        # stat piece-pairs in MM order: Gx pairs (0-4) on the sync ring ahead
        # of the image loads; Gy pairs (5-8) on the scalar ring, whose
        # triggers sit behind the ~2.7us ACT table load.
        stat_sb = const_pool.tile([TILE_K, 18, TILE_M], mm_dt)
        for j in range(5):
            nc.sync.dma_start(
                out=stat_sb[:, 2 * j : 2 * j + 2], in_=stat_dram[:, 2 * j : 2 * j + 2]
            )
        for j in range(5, 9):
            nc.scalar.dma_start(
                out=stat_sb[:, 2 * j : 2 * j + 2], in_=stat_dram[:, 2 * j : 2 * j + 2]
            )

        def big_tile(n, t):
            y0 = t * TILE_M
            # per-channel loads -> finer-grained MM/DMA pipelining. All loads
            # on the sync HWDGE ring, all stores on the scalar ring: measured
            # 287 GB/s vs 215 GB/s with loads+stores sharing a ring.
            its = []
            for c in range(3):
                itc = img_pool.tile(
                    [TILE_K, W], mm_dt, tag=f"it{c}", name=f"it{c}", bufs=6
                )
                nc.sync.dma_start(out=itc, in_=img_yx[n, y0 : y0 + TILE_K, c])
                its.append(itc)

            psx = psum_pool.tile([TILE_M, NW], f32, tag="psx", name="psx")
            psy = psum_pool.tile([TILE_M, NW], f32, tag="psy", name="psy")
            for g, ps in ((0, psx), (1, psy)):
                mmi = 0
                for c in range(3):
                    for dx in range(3):
                        i = (g * 3 + c) * 3 + dx
                        nc.tensor.matmul(
                            ps,
                            stat_sb[:, i, :],
                            its[c][:, dx : dx + NW],
                            start=(mmi == 0),
                            stop=(mmi == 8),
                        )
                        mmi += 1

            mag = _epilogue(nc, work_pool, psx, psy, TILE_M, f32)
            nc.scalar.dma_start(out=out[n, 1 + y0 : 1 + y0 + TILE_M, :], in_=mag)
            if t == 0:
                nc.scalar.dma_start(out=out[n, 0:1, :], in_=mag[0:1, :])

        def mini_tile():
            # last 6 valid rows (y' = 504..509) of all 4 images at once,
            # via a block-diagonal stationary
            mpsx = psum_pool.tile([MINI_M, NW], f32, tag="mpsx", bufs=1, name="mpsx")
            mpsy = psum_pool.tile([MINI_M, NW], f32, tag="mpsy", bufs=1, name="mpsy")
            for g, ps in ((0, mpsx), (1, mpsy)):
                mmi = 0
                for c in range(3):
                    for dx in range(3):
                        i = (g * 3 + c) * 3 + dx
                        nc.tensor.matmul(
                            ps,
                            stat_mini_sb[:, i, :],
                            mit[:, c, dx : dx + NW],
                            start=(mmi == 0),
                            stop=(mmi == 8),
                        )
                        mmi += 1
            mmag = _epilogue(nc, work_pool, mpsx, mpsy, MINI_M, f32)
            for n in range(N_PER_CORE):
                nc.scalar.dma_start(
                    out=out[n, H - 7 : H - 1, :], in_=mmag[n * 6 : n * 6 + 6]
                )
                nc.scalar.dma_start(
                    out=out[n, H - 1 : H, :], in_=mmag[n * 6 + 5 : n * 6 + 6]
                )

        mini_tile()
        for n in range(N_PER_CORE):
            for t in range(N_TILES):
                big_tile(n, t)


def _build_program():
    import concourse.bacc as bacc
    import concourse.mybir as mybir
    import concourse.tile as tile

    nc = bacc.Bacc(
        "TRN2",
        target_bir_lowering=False,
        debug=False,
        num_devices=N_CORES,
    )
    img = nc.dram_tensor(
        "img", [N_PER_CORE, 3, H, W], mybir.dt.float32r, kind="ExternalInput"
    ).ap()
    stat = nc.dram_tensor(
        "stat", [TILE_K, 18, TILE_M], mybir.dt.float32r, kind="ExternalInput"
    ).ap()
    stat_mini = nc.dram_tensor(
        "stat_mini", [MINI_K, 18, MINI_M], mybir.dt.float32r, kind="ExternalInput"
    ).ap()
    out = nc.dram_tensor(
        "out", [N_PER_CORE, H, W], mybir.dt.float32, kind="ExternalOutput"
    ).ap()

    with tile.TileContext(nc) as tc:
        _sobel_body(tc, out, img, stat, stat_mini)
    nc.compile()
    return nc


def kernel(img: np.ndarray, kx: np.ndarray, ky: np.ndarray) -> np.ndarray:
    global LAST_RESULTS
    from concourse.bass_utils import run_bass_kernel_spmd

    img = np.ascontiguousarray(np.asarray(img, dtype=np.float32))
    assert img.shape == (N_FULL, 3, H, W), img.shape
    stat, stat_mini = _build_stationaries(kx, ky)

    if "nc" not in _CACHE:
        _CACHE["nc"] = _build_program()
    nc = _CACHE["nc"]

    in_maps = [
        {
            "img": img[c * N_PER_CORE : (c + 1) * N_PER_CORE],
            "stat": stat,
            "stat_mini": stat_mini,
        }
        for c in range(N_CORES)
    ]
    trace = os.environ.get("SOBEL_TRACE", "0") == "1"
    res = run_bass_kernel_spmd(
        nc, in_maps, core_ids=list(range(N_CORES)), trace=trace
    )
    LAST_RESULTS = res
    out = np.concatenate([res.results[c]["out"] for c in range(N_CORES)], axis=0)
    return out.reshape(N_FULL, 1, H, W)


# revision 33
# speedup vs baseline: 1.2328x; 1.0470x over previous
"""Sobel filter Trainium2 Bass kernel.

Problem: img [32, 3, 512, 512] f32, kx/ky [1, 3, 3, 3] f32 (same 3x3 kernel
broadcast over the 3 input channels in the reference, but we honor arbitrary
values). Output [32, 1, 512, 512] f32:
    Gx = valid_conv3x3(img, kx), Gy = valid_conv3x3(img, ky)  -> [N,1,510,510]
    out = sqrt(Gx^2 + Gy^2) edge-padded by 1 back to [N,1,512,512]

Strategy (pure data parallel over 8 NeuronCores, 4 images per core):
  The 2D conv runs on the TensorEngine as sums of banded-Toeplitz matmuls.
  Partition dim = image rows (y). For each (channel c, x-shift dx) the 3-tap
  y-convolution is a banded [K=128, M=126] stationary matrix
  A[k, m] = w[c, k-m, dx]; the moving operand is the x-shifted image rows
  img[c, y0:y0+128, dx:dx+510]. Summing over (c, dx) for each of Gx/Gy is
  PSUM accumulation over 9 matmuls -> [126, 510] valid conv rows per PSUM
  tile. 4 row-tiles of 126 cover rows 0..503; the remaining 6 valid rows of
  ALL 4 images are computed by one extra "mini" tile with a block-diagonal
  [32, 24] stationary (4 blocks of [8 in-rows, 6 out-rows]).

  Matmul operands are float32r (full-rate fp32 matmul mode; plain float32
  streams at 1/4 rate). Loads use 128-partition DMAs (104-partition DMAs
  measured at 159 GB/s vs 286 GB/s for 128). Magnitude epilogue: squares on
  ScalarE (PSUM->SBUF), add on VectorE, sqrt on ScalarE; column edge padding
  in-SBUF, row edge padding via small extra stores.

The banded stationary matrices (built from kx/ky on host) are passed as
replicated input tensors.
"""

import os

import numpy as np

N_CORES = 8
N_FULL = 32          # full batch
N_PER_CORE = N_FULL // N_CORES
H = W = 512
TILE_K = 128         # input rows per full row-tile
TILE_M = 126         # valid output rows per full row-tile
N_TILES = 4          # 4 * 126 = 504 valid rows; remaining 6 via mini tile
NW = 510             # valid output columns
MINI_K = 8 * N_PER_CORE   # 4 images x 8 input rows
MINI_M = 6 * N_PER_CORE   # 4 images x 6 output rows

_CACHE: dict = {}
LAST_RESULTS = None  # BassKernelResults of the most recent run (for test.py)


def _build_stationaries(kx: np.ndarray, ky: np.ndarray):
    """Returns (stat [TILE_K, 18, TILE_M], stat_mini [MINI_K, 18, MINI_M]).
    Slice i=(g,c,dx) of stat is the banded matrix A[k, m] = kG[c, k-m, dx]
    for k-m in {0,1,2}; stat_mini is block-diagonal per image."""
    ks = (np.asarray(kx, np.float32), np.asarray(ky, np.float32))
    stat = np.zeros((18, TILE_K, TILE_M), np.float32)
    mini = np.zeros((18, MINI_K, MINI_M), np.float32)
    m = np.arange(TILE_M)
    mm = np.arange(6)
    i = 0
    for g in range(2):
        for c in range(3):
            for dx in range(3):
                for dy in range(3):
                    stat[i, m + dy, m] = ks[g][0, c, dy, dx]
                    for j in range(N_PER_CORE):
                        mini[i, j * 8 + mm + dy, j * 6 + mm] = ks[g][0, c, dy, dx]
                i += 1
    return (
        np.ascontiguousarray(stat.transpose(1, 0, 2)),
        np.ascontiguousarray(mini.transpose(1, 0, 2)),
    )


def _epilogue(nc, work_pool, psx, psy, rows, f32):
    """sqrt(psx^2 + psy^2) -> [rows, 512] SBUF tile with edge cols."""
    s = work_pool.tile([rows, W], f32, tag="s", name="s")
    s2 = work_pool.tile([rows, NW], f32, tag="s2", name="s2")
    nc.scalar.square(s[:, 1 : 1 + NW], psx)
    nc.scalar.square(s2, psy)
    nc.vector.tensor_add(s[:, 1 : 1 + NW], s[:, 1 : 1 + NW], s2)
    nc.vector.tensor_copy(s[:, 0:1], s[:, 1:2])
    nc.vector.tensor_copy(s[:, W - 1 : W], s[:, W - 2 : W - 1])
    mag = work_pool.tile([rows, W], f32, tag="mag", name="mag")
    nc.scalar.sqrt(mag, s)
    return mag


def _sobel_body(tc, out, img, stat_dram, stat_mini_dram):
    import concourse.mybir as mybir

    nc = tc.nc
    f32 = mybir.dt.float32
    mm_dt = mybir.dt.float32r

    img_yx = img.rearrange("n c y x -> n y c x")

    with (
        tc.tile_pool(name="const", bufs=1) as const_pool,
        tc.tile_pool(name="imgs", bufs=3) as img_pool,
        tc.tile_pool(name="work", bufs=3) as work_pool,
        tc.tile_pool(name="psum", bufs=2, space="PSUM") as psum_pool,
    ):
        # Load order is tuned so the PE can start early: the tiny mini-tile
        # inputs go first on the sync ring, so the mini matmuls do useful
        # work (and ramp the PE clock) while the big stat matrix and first
        # image tiles stream in behind them.
        stat_mini_sb = const_pool.tile([MINI_K, 18, MINI_M], mm_dt)
        nc.sync.dma_start(out=stat_mini_sb, in_=stat_mini_dram)
        # per-channel 32-partition DMAs (narrower DMAs steal
        # disproportionate SDMA-engine time)
        mit = img_pool.tile([MINI_K, 3, W], mm_dt, tag="mit", bufs=1)
        for c in range(3):
            nc.sync.dma_start(out=mit[:, c, :], in_=img_yx[:, H - 8 : H, c])
        # stat piece-pairs in MM order: Gx pairs (0-4) on the sync ring ahead
        # of the image loads; Gy pairs (5-8) on the scalar ring, whose
        # triggers sit behind the ~2.7us ACT table load.
        stat_sb = const_pool.tile([TILE_K, 18, TILE_M], mm_dt)
        for j in range(5):
            nc.sync.dma_start(
                out=stat_sb[:, 2 * j : 2 * j + 2], in_=stat_dram[:, 2 * j : 2 * j + 2]
            )
        for j in range(5, 9):
            nc.scalar.dma_start(
                out=stat_sb[:, 2 * j : 2 * j + 2], in_=stat_dram[:, 2 * j : 2 * j + 2]
            )

        def big_tile(n, t):
            y0 = t * TILE_M
            # per-channel loads -> finer-grained MM/DMA pipelining. All loads
            # on the sync HWDGE ring, all stores on the scalar ring: measured
            # 287 GB/s vs 215 GB/s with loads+stores sharing a ring.
            its = []
            for c in range(3):
                itc = img_pool.tile(
                    [TILE_K, W], mm_dt, tag=f"it{c}", name=f"it{c}", bufs=6
                )
                nc.sync.dma_start(out=itc, in_=img_yx[n, y0 : y0 + TILE_K, c])
                its.append(itc)

            psx = psum_pool.tile([TILE_M, NW], f32, tag="psx", name="psx")
            psy = psum_pool.tile([TILE_M, NW], f32, tag="psy", name="psy")
            for g, ps in ((0, psx), (1, psy)):
                mmi = 0
                for c in range(3):
                    for dx in range(3):
                        i = (g * 3 + c) * 3 + dx
                        nc.tensor.matmul(
                            ps,
                            stat_sb[:, i, :],
                            its[c][:, dx : dx + NW],
                            start=(mmi == 0),
                            stop=(mmi == 8),
                        )
                        mmi += 1

            mag = _epilogue(nc, work_pool, psx, psy, TILE_M, f32)
            nc.scalar.dma_start(out=out[n, 1 + y0 : 1 + y0 + TILE_M, :], in_=mag)
            if t == 0:
                nc.scalar.dma_start(out=out[n, 0:1, :], in_=mag[0:1, :])

        def mini_tile():
            # last 6 valid rows (y' = 504..509) of all 4 images at once,
            # via a block-diagonal stationary
            mpsx = psum_pool.tile([MINI_M, NW], f32, tag="mpsx", bufs=1, name="mpsx")
            mpsy = psum_pool.tile([MINI_M, NW], f32, tag="mpsy", bufs=1, name="mpsy")
            for g, ps in ((0, mpsx), (1, mpsy)):
                mmi = 0
                for c in range(3):
                    for dx in range(3):
                        i = (g * 3 + c) * 3 + dx
                        nc.tensor.matmul(
                            ps,
                            stat_mini_sb[:, i, :],
                            mit[:, c, dx : dx + NW],
                            start=(mmi == 0),
                            stop=(mmi == 8),
                        )
                        mmi += 1
            mmag = _epilogue(nc, work_pool, mpsx, mpsy, MINI_M, f32)
            for n in range(N_PER_CORE):
                nc.scalar.dma_start(
                    out=out[n, H - 7 : H - 1, :], in_=mmag[n * 6 : n * 6 + 6]
                )
                nc.scalar.dma_start(
                    out=out[n, H - 1 : H, :], in_=mmag[n * 6 + 5 : n * 6 + 6]
                )

        mini_tile()
        for n in range(N_PER_CORE):
            for t in range(N_TILES):
                big_tile(n, t)


def _build_program():
    import concourse.bacc as bacc
    import concourse.mybir as mybir
    import concourse.tile as tile

    nc = bacc.Bacc(
        "TRN2",
        target_bir_lowering=False,
        debug=False,
        num_devices=N_CORES,
    )
    img = nc.dram_tensor(
        "img", [N_PER_CORE, 3, H, W], mybir.dt.float32r, kind="ExternalInput"
    ).ap()
    stat = nc.dram_tensor(
        "stat", [TILE_K, 18, TILE_M], mybir.dt.float32r, kind="ExternalInput"
    ).ap()
    stat_mini = nc.dram_tensor(
        "stat_mini", [MINI_K, 18, MINI_M], mybir.dt.float32r, kind="ExternalInput"
    ).ap()
    out = nc.dram_tensor(
        "out", [N_PER_CORE, H, W], mybir.dt.float32, kind="ExternalOutput"
    ).ap()

    with tile.TileContext(nc) as tc:
        _sobel_body(tc, out, img, stat, stat_mini)
    nc.compile()
    return nc


# ---------------------------------------------------------------------------
# Separable fast path.
#
# The reference Sobel kernels are rank-1: kG[c, dy, dx] = a[c] * b[dy] * g[dx]
# (channel-proportional and separable). Then
#   G = Xconv_g( Sum_c a[c] * Yconv_b(img_c) )
# The y-conv + channel sum is 3 accumulating banded matmuls per PSUM tile
# (instead of 9), and the 3-tap x-conv is cheap elementwise work spread over
# DVE / GPSIMD / ScalarE. PE work drops 3x; the kernel becomes DMA-bound.
# ---------------------------------------------------------------------------


def _rank1_decompose(k: np.ndarray):
    """k [1,3,3,3] -> (a[3], b[3], g[3]) with k[0,c,dy,dx] = a_c b_dy g_dx,
    or None if not (numerically exactly) rank-1."""
    k2 = np.asarray(k, np.float64)[0]
    scale = np.abs(k2).max()
    if scale == 0:
        return None
    u, s, vt = np.linalg.svd(k2.reshape(3, 9), full_matrices=False)
    a = u[:, 0] * s[0]
    v = vt[0].reshape(3, 3)
    u2, s2, vt2 = np.linalg.svd(v, full_matrices=False)
    b = u2[:, 0] * s2[0]
    g = vt2[0]
    rec = np.einsum("c,y,x->cyx", a, b, g)
    if np.abs(rec - k2).max() > 1e-6 * scale:
        return None
    # normalize so the largest |g| tap is exactly 1 (its x-conv op is free)
    gm = g[np.argmax(np.abs(g))]
    g = g / gm
    a = a * gm
    return a.astype(np.float64), b.astype(np.float64), g.astype(np.float64)


def _build_stationaries_sep(ax, bx, ay, by):
    """stat_sep [TILE_K, 6, TILE_M] (j = G*3+c: band(a_G[c]*b_G)) and
    stat_sep_mini [MINI_K, 6, MINI_M] block-diagonal per image."""
    stat = np.zeros((6, TILE_K, TILE_M), np.float32)
    mini = np.zeros((6, MINI_K, MINI_M), np.float32)
    m = np.arange(TILE_M)
    mm = np.arange(6)
    for gi, (a, b) in enumerate(((ax, bx), (ay, by))):
        for c in range(3):
            j = gi * 3 + c
            for dy in range(3):
                w = np.float32(a[c] * b[dy])
                stat[j, m + dy, m] = w
                for im in range(N_PER_CORE):
                    mini[j, im * 8 + mm + dy, im * 6 + mm] = w
    return (
        np.ascontiguousarray(stat.transpose(1, 0, 2)),
        np.ascontiguousarray(mini.transpose(1, 0, 2)),
    )


def _emit_xconv(nc, work_pool, S, taps, rows, f32, name, mid_engine):
    """out[rows, NW] = sum_dx taps[dx] * S[:, dx:dx+NW]; zero taps skipped.
    Chain ops emitted on DVE except one middle op on `mid_engine`."""
    import concourse.mybir as mybir

    L = [(float(taps[dx]), dx) for dx in range(3) if taps[dx] != 0.0]
    assert L
    outt = work_pool.tile([rows, NW], f32, tag=name, name=name)
    acc = None
    for idx, (w, dx) in enumerate(L):
        src = S[:, dx : dx + NW]  # S may be PSUM: one PSUM operand per op
        if acc is None:
            nc.vector.tensor_scalar_mul(outt, src, w)
        else:
            nc.vector.scalar_tensor_tensor(
                outt, src, w, acc, mybir.AluOpType.mult, mybir.AluOpType.add
            )
        acc = outt
    return outt


def _epilogue_sep(nc, work_pool, ps1, ps2, gx_taps, gy_taps, rows, f32):
    """x-convs + magnitude from the two y-conv PSUM tiles. The x-conv chains
    read the PSUM tiles directly (one PSUM operand per op), avoiding
    PSUM->SBUF staging copies."""
    gx = _emit_xconv(nc, work_pool, ps1, gx_taps, rows, f32, "gx", nc.vector)
    gy = _emit_xconv(nc, work_pool, ps2, gy_taps, rows, f32, "gy", nc.vector)
    s = work_pool.tile([rows, W], f32, tag="s", name="s")
    s2 = work_pool.tile([rows, NW], f32, tag="s2", name="s2")
    nc.scalar.square(s[:, 1 : 1 + NW], gx)
    nc.scalar.square(s2, gy)
    nc.gpsimd.tensor_add(s[:, 1 : 1 + NW], s[:, 1 : 1 + NW], s2)
    nc.vector.tensor_copy(s[:, 0:1], s[:, 1:2])
    nc.vector.tensor_copy(s[:, W - 1 : W], s[:, W - 2 : W - 1])
    mag = work_pool.tile([rows, W], f32, tag="mag", name="mag")
    nc.scalar.sqrt(mag, s)
    return mag


def _sobel_body_sep(tc, out, img, stat_dram, stat_mini_dram, gx_taps, gy_taps):
    import concourse.mybir as mybir

    nc = tc.nc
    f32 = mybir.dt.float32
    mm_dt = mybir.dt.float32r

    img_yx = img.rearrange("n c y x -> n y c x")

    with (
        tc.tile_pool(name="const", bufs=1) as const_pool,
        tc.tile_pool(name="imgs", bufs=3) as img_pool,
        tc.tile_pool(name="work", bufs=3) as work_pool,
        tc.tile_pool(name="psum", bufs=2, space="PSUM") as psum_pool,
    ):
        stat_mini_sb = const_pool.tile([MINI_K, 6, MINI_M], mm_dt)
        nc.sync.dma_start(out=stat_mini_sb, in_=stat_mini_dram)
        mit = img_pool.tile([MINI_K, 3, W], mm_dt, tag="mit", bufs=1)
        for c in range(3):
            nc.sync.dma_start(out=mit[:, c, :], in_=img_yx[:, H - 8 : H, c])
        stat_sb = const_pool.tile([TILE_K, 6, TILE_M], mm_dt)
        nc.sync.dma_start(out=stat_sb, in_=stat_dram)

        def run_groups(stat_t, src_fn, rows_m, pool_tags):
            ps1 = psum_pool.tile([rows_m, W], f32, tag=pool_tags[0], name=pool_tags[0],
                                 bufs=1 if rows_m == MINI_M else None)
            ps2 = psum_pool.tile([rows_m, W], f32, tag=pool_tags[1], name=pool_tags[1],
                                 bufs=1 if rows_m == MINI_M else None)
            for gi, ps in ((0, ps1), (1, ps2)):
                for c in range(3):
                    nc.tensor.matmul(
                        ps,
                        stat_t[:, gi * 3 + c, :],
                        src_fn(c),
                        start=(c == 0),
                        stop=(c == 2),
                    )
            return ps1, ps2

        # mini tile first (tiny deps -> PE starts early)
        mps1, mps2 = run_groups(
            stat_mini_sb, lambda c: mit[:, c, :], MINI_M, ("mps1", "mps2")
        )
        mmag = _epilogue_sep(nc, work_pool, mps1, mps2, gx_taps, gy_taps, MINI_M, f32)
        for n in range(N_PER_CORE):
            nc.scalar.dma_start(
                out=out[n, H - 7 : H - 1, :], in_=mmag[n * 6 : n * 6 + 6]
            )
            nc.scalar.dma_start(
                out=out[n, H - 1 : H, :], in_=mmag[n * 6 + 5 : n * 6 + 6]
            )

        for n in range(N_PER_CORE):
            for t in range(N_TILES):
                y0 = t * TILE_M
                its = []
                for c in range(3):
                    itc = img_pool.tile(
                        [TILE_K, W], mm_dt, tag=f"it{c}", name=f"it{c}", bufs=6
                    )
                    nc.sync.dma_start(out=itc, in_=img_yx[n, y0 : y0 + TILE_K, c])
                    its.append(itc)
                ps1, ps2 = run_groups(
                    stat_sb, lambda c: its[c], TILE_M, ("ps1", "ps2")
                )
                mag = _epilogue_sep(
                    nc, work_pool, ps1, ps2, gx_taps, gy_taps, TILE_M, f32
                )
                nc.scalar.dma_start(
                    out=out[n, 1 + y0 : 1 + y0 + TILE_M, :], in_=mag
                )
                if t == 0:
                    nc.scalar.dma_start(out=out[n, 0:1, :], in_=mag[0:1, :])


def _build_program_sep(gx_taps, gy_taps):
    import concourse.bacc as bacc
    import concourse.mybir as mybir
    import concourse.tile as tile

    nc = bacc.Bacc(
        "TRN2", target_bir_lowering=False, debug=False, num_devices=N_CORES
    )
    img = nc.dram_tensor(
        "img", [N_PER_CORE, 3, H, W], mybir.dt.float32r, kind="ExternalInput"
    ).ap()
    stat = nc.dram_tensor(
        "stat", [TILE_K, 6, TILE_M], mybir.dt.float32r, kind="ExternalInput"
    ).ap()
    stat_mini = nc.dram_tensor(
        "stat_mini", [MINI_K, 6, MINI_M], mybir.dt.float32r, kind="ExternalInput"
    ).ap()
    out = nc.dram_tensor(
        "out", [N_PER_CORE, H, W], mybir.dt.float32, kind="ExternalOutput"
    ).ap()
    with tile.TileContext(nc) as tc:
        _sobel_body_sep(tc, out, img, stat, stat_mini, gx_taps, gy_taps)
    nc.compile()
    return nc


def _run(nc, in_maps):
    global LAST_RESULTS
    from concourse.bass_utils import run_bass_kernel_spmd

    trace = os.environ.get("SOBEL_TRACE", "0") == "1"
    res = run_bass_kernel_spmd(
        nc, in_maps, core_ids=list(range(N_CORES)), trace=trace
    )
    LAST_RESULTS = res
    out = np.concatenate([res.results[c]["out"] for c in range(N_CORES)], axis=0)
    return out.reshape(N_FULL, 1, H, W)


def kernel(img: np.ndarray, kx: np.ndarray, ky: np.ndarray) -> np.ndarray:
    img = np.ascontiguousarray(np.asarray(img, dtype=np.float32))
    assert img.shape == (N_FULL, 3, H, W), img.shape

    dx_ = _rank1_decompose(kx) if os.environ.get("SOBEL_NO_SEP", "0") != "1" else None
    dy_ = _rank1_decompose(ky) if dx_ is not None else None
    if dx_ is not None and dy_ is not None:
        (axc, bx, gx_t), (ayc, by, gy_t) = dx_, dy_
        stat, stat_mini = _build_stationaries_sep(axc, bx, ayc, by)
        key = ("sep", tuple(np.round(gx_t, 12)), tuple(np.round(gy_t, 12)))
        if key not in _CACHE:
            _CACHE[key] = _build_program_sep(tuple(gx_t), tuple(gy_t))
        nc = _CACHE[key]
    else:
        stat, stat_mini = _build_stationaries(kx, ky)
        if "gen" not in _CACHE:
            _CACHE["gen"] = _build_program()
        nc = _CACHE["gen"]

    in_maps = [
        {
            "img": img[c * N_PER_CORE : (c + 1) * N_PER_CORE],
            "stat": stat,
            "stat_mini": stat_mini,
        }
        for c in range(N_CORES)
    ]
    return _run(nc, in_maps)


# revision 35
# speedup vs baseline: 1.2825x; 1.0403x over previous
"""Sobel filter Trainium2 Bass kernel.

Problem: img [32, 3, 512, 512] f32, kx/ky [1, 3, 3, 3] f32 (same 3x3 kernel
broadcast over the 3 input channels in the reference, but we honor arbitrary
values). Output [32, 1, 512, 512] f32:
    Gx = valid_conv3x3(img, kx), Gy = valid_conv3x3(img, ky)  -> [N,1,510,510]
    out = sqrt(Gx^2 + Gy^2) edge-padded by 1 back to [N,1,512,512]

Strategy (pure data parallel over 8 NeuronCores, 4 images per core):
  The 2D conv runs on the TensorEngine as sums of banded-Toeplitz matmuls.
  Partition dim = image rows (y). For each (channel c, x-shift dx) the 3-tap
  y-convolution is a banded [K=128, M=126] stationary matrix
  A[k, m] = w[c, k-m, dx]; the moving operand is the x-shifted image rows
  img[c, y0:y0+128, dx:dx+510]. Summing over (c, dx) for each of Gx/Gy is
  PSUM accumulation over 9 matmuls -> [126, 510] valid conv rows per PSUM
  tile. 4 row-tiles of 126 cover rows 0..503; the remaining 6 valid rows of
  ALL 4 images are computed by one extra "mini" tile with a block-diagonal
  [32, 24] stationary (4 blocks of [8 in-rows, 6 out-rows]).

  Matmul operands are float32r (full-rate fp32 matmul mode; plain float32
  streams at 1/4 rate). Loads use 128-partition DMAs (104-partition DMAs
  measured at 159 GB/s vs 286 GB/s for 128). Magnitude epilogue: squares on
  ScalarE (PSUM->SBUF), add on VectorE, sqrt on ScalarE; column edge padding
  in-SBUF, row edge padding via small extra stores.

The banded stationary matrices (built from kx/ky on host) are passed as
replicated input tensors.
"""

import os

import numpy as np

N_CORES = 8
N_FULL = 32          # full batch
N_PER_CORE = N_FULL // N_CORES
H = W = 512
TILE_K = 128         # input rows per full row-tile
TILE_M = 126         # valid output rows per full row-tile
N_TILES = 4          # 4 * 126 = 504 valid rows; remaining 6 via mini tile
NW = 510             # valid output columns
MINI_K = 8 * N_PER_CORE   # 4 images x 8 input rows
MINI_M = 6 * N_PER_CORE   # 4 images x 6 output rows

_CACHE: dict = {}
LAST_RESULTS = None  # BassKernelResults of the most recent run (for test.py)


def _build_stationaries(kx: np.ndarray, ky: np.ndarray):
    """Returns (stat [TILE_K, 18, TILE_M], stat_mini [MINI_K, 18, MINI_M]).
    Slice i=(g,c,dx) of stat is the banded matrix A[k, m] = kG[c, k-m, dx]
    for k-m in {0,1,2}; stat_mini is block-diagonal per image."""
    ks = (np.asarray(kx, np.float32), np.asarray(ky, np.float32))
    stat = np.zeros((18, TILE_K, TILE_M), np.float32)
    mini = np.zeros((18, MINI_K, MINI_M), np.float32)
    m = np.arange(TILE_M)
    mm = np.arange(6)
    i = 0
    for g in range(2):
        for c in range(3):
            for dx in range(3):
                for dy in range(3):
                    stat[i, m + dy, m] = ks[g][0, c, dy, dx]
                    for j in range(N_PER_CORE):
                        mini[i, j * 8 + mm + dy, j * 6 + mm] = ks[g][0, c, dy, dx]
                i += 1
    return (
        np.ascontiguousarray(stat.transpose(1, 0, 2)),
        np.ascontiguousarray(mini.transpose(1, 0, 2)),
    )


def _epilogue(nc, work_pool, psx, psy, rows, f32):
    """sqrt(psx^2 + psy^2) -> [rows, 512] SBUF tile with edge cols."""
    s = work_pool.tile([rows, W], f32, tag="s", name="s")
    s2 = work_pool.tile([rows, NW], f32, tag="s2", name="s2")
    nc.scalar.square(s[:, 1 : 1 + NW], psx)
    nc.scalar.square(s2, psy)
    nc.vector.tensor_add(s[:, 1 : 1 + NW], s[:, 1 : 1 + NW], s2)
    nc.vector.tensor_copy(s[:, 0:1], s[:, 1:2])
    nc.vector.tensor_copy(s[:, W - 1 : W], s[:, W - 2 : W - 1])
    mag = work_pool.tile([rows, W], f32, tag="mag", name="mag")
    nc.scalar.sqrt(mag, s)
    return mag


def _sobel_body(tc, out, img, stat_dram, stat_mini_dram):
    import concourse.mybir as mybir

    nc = tc.nc
    f32 = mybir.dt.float32
    mm_dt = mybir.dt.float32r

    img_yx = img.rearrange("n c y x -> n y c x")

    with (
        tc.tile_pool(name="const", bufs=1) as const_pool,
        tc.tile_pool(name="imgs", bufs=3) as img_pool,
        tc.tile_pool(name="work", bufs=3) as work_pool,
        tc.tile_pool(name="psum", bufs=2, space="PSUM") as psum_pool,
    ):
        # Load order is tuned so the PE can start early: the tiny mini-tile
        # inputs go first on the sync ring, so the mini matmuls do useful
        # work (and ramp the PE clock) while the big stat matrix and first
        # image tiles stream in behind them.
        stat_mini_sb = const_pool.tile([MINI_K, 18, MINI_M], mm_dt)
        nc.sync.dma_start(out=stat_mini_sb, in_=stat_mini_dram)
        # per-channel 32-partition DMAs (narrower DMAs steal
        # disproportionate SDMA-engine time)
        mit = img_pool.tile([MINI_K, 3, W], mm_dt, tag="mit", bufs=1)
        for c in range(3):
            nc.sync.dma_start(out=mit[:, c, :], in_=img_yx[:, H - 8 : H, c])
        # stat piece-pairs in MM order: Gx pairs (0-4) on the sync ring ahead
        # of the image loads; Gy pairs (5-8) on the scalar ring, whose
        # triggers sit behind the ~2.7us ACT table load.
        stat_sb = const_pool.tile([TILE_K, 18, TILE_M], mm_dt)
        for j in range(5):
            nc.sync.dma_start(
                out=stat_sb[:, 2 * j : 2 * j + 2], in_=stat_dram[:, 2 * j : 2 * j + 2]
            )
        for j in range(5, 9):
            nc.scalar.dma_start(
                out=stat_sb[:, 2 * j : 2 * j + 2], in_=stat_dram[:, 2 * j : 2 * j + 2]
            )

        def big_tile(n, t):
            y0 = t * TILE_M
            # per-channel loads -> finer-grained MM/DMA pipelining. All loads
            # on the sync HWDGE ring, all stores on the scalar ring: measured
            # 287 GB/s vs 215 GB/s with loads+stores sharing a ring.
            its = []
            for c in range(3):
                itc = img_pool.tile(
                    [TILE_K, W], mm_dt, tag=f"it{c}", name=f"it{c}", bufs=6
                )
                nc.sync.dma_start(out=itc, in_=img_yx[n, y0 : y0 + TILE_K, c])
                its.append(itc)

            psx = psum_pool.tile([TILE_M, NW], f32, tag="psx", name="psx")
            psy = psum_pool.tile([TILE_M, NW], f32, tag="psy", name="psy")
            for g, ps in ((0, psx), (1, psy)):
                mmi = 0
                for c in range(3):
                    for dx in range(3):
                        i = (g * 3 + c) * 3 + dx
                        nc.tensor.matmul(
                            ps,
                            stat_sb[:, i, :],
                            its[c][:, dx : dx + NW],
                            start=(mmi == 0),
                            stop=(mmi == 8),
                        )
                        mmi += 1

            mag = _epilogue(nc, work_pool, psx, psy, TILE_M, f32)
            nc.scalar.dma_start(out=out[n, 1 + y0 : 1 + y0 + TILE_M, :], in_=mag)
            if t == 0:
                nc.scalar.dma_start(out=out[n, 0:1, :], in_=mag[0:1, :])

        def mini_tile():
            # last 6 valid rows (y' = 504..509) of all 4 images at once,
            # via a block-diagonal stationary
            mpsx = psum_pool.tile([MINI_M, NW], f32, tag="mpsx", bufs=1, name="mpsx")
            mpsy = psum_pool.tile([MINI_M, NW], f32, tag="mpsy", bufs=1, name="mpsy")
            for g, ps in ((0, mpsx), (1, mpsy)):
                mmi = 0
                for c in range(3):
                    for dx in range(3):
                        i = (g * 3 + c) * 3 + dx
                        nc.tensor.matmul(
                            ps,
                            stat_mini_sb[:, i, :],
                            mit[:, c, dx : dx + NW],
                            start=(mmi == 0),
                            stop=(mmi == 8),
                        )
                        mmi += 1
            mmag = _epilogue(nc, work_pool, mpsx, mpsy, MINI_M, f32)
            for n in range(N_PER_CORE):
                nc.scalar.dma_start(
                    out=out[n, H - 7 : H - 1, :], in_=mmag[n * 6 : n * 6 + 6]
                )
                nc.scalar.dma_start(
                    out=out[n, H - 1 : H, :], in_=mmag[n * 6 + 5 : n * 6 + 6]
                )

        mini_tile()
        for n in range(N_PER_CORE):
            for t in range(N_TILES):
                big_tile(n, t)


def _build_program():
    import concourse.bacc as bacc
    import concourse.mybir as mybir
    import concourse.tile as tile

    nc = bacc.Bacc(
        "TRN2",
        target_bir_lowering=False,
        debug=False,
        num_devices=N_CORES,
    )
    img = nc.dram_tensor(
        "img", [N_PER_CORE, 3, H, W], mybir.dt.float32r, kind="ExternalInput"
    ).ap()
    stat = nc.dram_tensor(
        "stat", [TILE_K, 18, TILE_M], mybir.dt.float32r, kind="ExternalInput"
    ).ap()
    stat_mini = nc.dram_tensor(
        "stat_mini", [MINI_K, 18, MINI_M], mybir.dt.float32r, kind="ExternalInput"
    ).ap()
    out = nc.dram_tensor(
        "out", [N_PER_CORE, H, W], mybir.dt.float32, kind="ExternalOutput"
    ).ap()

    with tile.TileContext(nc) as tc:
        _sobel_body(tc, out, img, stat, stat_mini)
    nc.compile()
    return nc


# ---------------------------------------------------------------------------
# Separable fast path.
#
# The reference Sobel kernels are rank-1: kG[c, dy, dx] = a[c] * b[dy] * g[dx]
# (channel-proportional and separable). Then
#   G = Xconv_g( Sum_c a[c] * Yconv_b(img_c) )
# The y-conv + channel sum is 3 accumulating banded matmuls per PSUM tile
# (instead of 9), and the 3-tap x-conv is cheap elementwise work spread over
# DVE / GPSIMD / ScalarE. PE work drops 3x; the kernel becomes DMA-bound.
# ---------------------------------------------------------------------------


def _rank1_decompose(k: np.ndarray):
    """k [1,3,3,3] -> (a[3], b[3], g[3]) with k[0,c,dy,dx] = a_c b_dy g_dx,
    or None if not (numerically exactly) rank-1."""
    k2 = np.asarray(k, np.float64)[0]
    scale = np.abs(k2).max()
    if scale == 0:
        return None
    u, s, vt = np.linalg.svd(k2.reshape(3, 9), full_matrices=False)
    a = u[:, 0] * s[0]
    v = vt[0].reshape(3, 3)
    u2, s2, vt2 = np.linalg.svd(v, full_matrices=False)
    b = u2[:, 0] * s2[0]
    g = vt2[0]
    rec = np.einsum("c,y,x->cyx", a, b, g)
    if np.abs(rec - k2).max() > 1e-6 * scale:
        return None
    # normalize so the largest |g| tap is exactly 1 (its x-conv op is free)
    gm = g[np.argmax(np.abs(g))]
    g = g / gm
    a = a * gm
    return a.astype(np.float64), b.astype(np.float64), g.astype(np.float64)


def _build_stationaries_sep(ax, bx, ay, by):
    """stat_sep [TILE_K, 6, TILE_M] (j = G*3+c: band(a_G[c]*b_G)) and
    stat_sep_mini [MINI_K, 6, MINI_M] block-diagonal per image."""
    stat = np.zeros((6, TILE_K, TILE_M), np.float32)
    mini = np.zeros((6, MINI_K, MINI_M), np.float32)
    m = np.arange(TILE_M)
    mm = np.arange(6)
    for gi, (a, b) in enumerate(((ax, bx), (ay, by))):
        for c in range(3):
            j = gi * 3 + c
            for dy in range(3):
                w = np.float32(a[c] * b[dy])
                stat[j, m + dy, m] = w
                for im in range(N_PER_CORE):
                    mini[j, im * 8 + mm + dy, im * 6 + mm] = w
    return (
        np.ascontiguousarray(stat.transpose(1, 0, 2)),
        np.ascontiguousarray(mini.transpose(1, 0, 2)),
    )


def _emit_xconv(nc, work_pool, S, taps, rows, f32, name, first_on_act=False):
    """out[rows, NW] = sum_dx taps[dx] * S[:, dx:dx+NW]; zero taps skipped.
    Chain ops on DVE; optionally the first (scale-copy) op on ScalarE to
    offload DVE."""
    import concourse.mybir as mybir

    L = [(float(taps[dx]), dx) for dx in range(3) if taps[dx] != 0.0]
    assert L
    outt = work_pool.tile([rows, NW], f32, tag=name, name=name)
    acc = None
    for idx, (w, dx) in enumerate(L):
        src = S[:, dx : dx + NW]  # S may be PSUM: one PSUM operand per op
        if acc is None:
            if first_on_act and len(L) > 1:
                nc.scalar.mul(outt, src, w)
            else:
                nc.vector.tensor_scalar_mul(outt, src, w)
        else:
            nc.vector.scalar_tensor_tensor(
                outt, src, w, acc, mybir.AluOpType.mult, mybir.AluOpType.add
            )
        acc = outt
    return outt


def _epilogue_sep(nc, work_pool, ps1, ps2, gx_taps, gy_taps, rows, f32):
    """x-convs + magnitude from the two y-conv PSUM tiles. The x-conv chains
    read the PSUM tiles directly (one PSUM operand per op), avoiding
    PSUM->SBUF staging copies."""
    gx = _emit_xconv(nc, work_pool, ps1, gx_taps, rows, f32, "gx")
    gy = _emit_xconv(nc, work_pool, ps2, gy_taps, rows, f32, "gy")
    s = work_pool.tile([rows, W], f32, tag="s", name="s")
    s2 = work_pool.tile([rows, NW], f32, tag="s2", name="s2")
    nc.scalar.square(s[:, 1 : 1 + NW], gx)
    nc.scalar.square(s2, gy)
    nc.gpsimd.tensor_add(s[:, 1 : 1 + NW], s[:, 1 : 1 + NW], s2)
    nc.vector.tensor_copy(s[:, 0:1], s[:, 1:2])
    nc.vector.tensor_copy(s[:, W - 1 : W], s[:, W - 2 : W - 1])
    mag = work_pool.tile([rows, W], f32, tag="mag", name="mag")
    nc.scalar.sqrt(mag, s)
    return mag


def _sobel_body_sep(tc, out, img, stat_dram, stat_mini_dram, gx_taps, gy_taps):
    import concourse.mybir as mybir

    nc = tc.nc
    f32 = mybir.dt.float32
    mm_dt = mybir.dt.float32r

    img_yx = img.rearrange("n c y x -> n y c x")

    with (
        tc.tile_pool(name="const", bufs=1) as const_pool,
        tc.tile_pool(name="imgs", bufs=3) as img_pool,
        tc.tile_pool(name="work", bufs=3) as work_pool,
        tc.tile_pool(name="psum", bufs=2, space="PSUM") as psum_pool,
    ):
        stat_mini_sb = const_pool.tile([MINI_K, 6, MINI_M], mm_dt)
        nc.sync.dma_start(out=stat_mini_sb, in_=stat_mini_dram)
        mit = img_pool.tile([MINI_K, 3, W], mm_dt, tag="mit", bufs=1)
        for c in range(3):
            nc.sync.dma_start(out=mit[:, c, :], in_=img_yx[:, H - 8 : H, c])
        stat_sb = const_pool.tile([TILE_K, 6, TILE_M], mm_dt)
        nc.sync.dma_start(out=stat_sb, in_=stat_dram)

        def run_groups(stat_t, src_fn, rows_m, pool_tags):
            ps1 = psum_pool.tile([rows_m, W], f32, tag=pool_tags[0], name=pool_tags[0],
                                 bufs=1 if rows_m == MINI_M else None)
            ps2 = psum_pool.tile([rows_m, W], f32, tag=pool_tags[1], name=pool_tags[1],
                                 bufs=1 if rows_m == MINI_M else None)
            for gi, ps in ((0, ps1), (1, ps2)):
                for c in range(3):
                    nc.tensor.matmul(
                        ps,
                        stat_t[:, gi * 3 + c, :],
                        src_fn(c),
                        start=(c == 0),
                        stop=(c == 2),
                    )
            return ps1, ps2

        # mini tile first (tiny deps -> PE starts early)
        mps1, mps2 = run_groups(
            stat_mini_sb, lambda c: mit[:, c, :], MINI_M, ("mps1", "mps2")
        )
        mmag = _epilogue_sep(nc, work_pool, mps1, mps2, gx_taps, gy_taps, MINI_M, f32)
        for n in range(N_PER_CORE):
            nc.scalar.dma_start(
                out=out[n, H - 7 : H - 1, :], in_=mmag[n * 6 : n * 6 + 6]
            )
            nc.scalar.dma_start(
                out=out[n, H - 1 : H, :], in_=mmag[n * 6 + 5 : n * 6 + 6]
            )

        for n in range(N_PER_CORE):
            for t in range(N_TILES):
                y0 = t * TILE_M
                its = []
                for c in range(3):
                    itc = img_pool.tile(
                        [TILE_K, W], mm_dt, tag=f"it{c}", name=f"it{c}", bufs=6
                    )
                    nc.sync.dma_start(out=itc, in_=img_yx[n, y0 : y0 + TILE_K, c])
                    its.append(itc)
                ps1, ps2 = run_groups(
                    stat_sb, lambda c: its[c], TILE_M, ("ps1", "ps2")
                )
                mag = _epilogue_sep(
                    nc, work_pool, ps1, ps2, gx_taps, gy_taps, TILE_M, f32
                )
                nc.scalar.dma_start(
                    out=out[n, 1 + y0 : 1 + y0 + TILE_M, :], in_=mag
                )
                if t == 0:
                    nc.scalar.dma_start(out=out[n, 0:1, :], in_=mag[0:1, :])


def _build_program_sep(gx_taps, gy_taps):
    import concourse.bacc as bacc
    import concourse.mybir as mybir
    import concourse.tile as tile

    nc = bacc.Bacc(
        "TRN2", target_bir_lowering=False, debug=False, num_devices=N_CORES
    )
    img = nc.dram_tensor(
        "img", [N_PER_CORE, 3, H, W], mybir.dt.float32r, kind="ExternalInput"
    ).ap()
    stat = nc.dram_tensor(
        "stat", [TILE_K, 6, TILE_M], mybir.dt.float32r, kind="ExternalInput"
    ).ap()
    stat_mini = nc.dram_tensor(
        "stat_mini", [MINI_K, 6, MINI_M], mybir.dt.float32r, kind="ExternalInput"
    ).ap()
    out = nc.dram_tensor(
        "out", [N_PER_CORE, H, W], mybir.dt.float32, kind="ExternalOutput"
    ).ap()
    with tile.TileContext(nc) as tc:
        _sobel_body_sep(tc, out, img, stat, stat_mini, gx_taps, gy_taps)
    nc.compile()
    return nc


def _run(nc, in_maps):
    global LAST_RESULTS
    from concourse.bass_utils import run_bass_kernel_spmd

    trace = os.environ.get("SOBEL_TRACE", "0") == "1"
    res = run_bass_kernel_spmd(
        nc, in_maps, core_ids=list(range(N_CORES)), trace=trace
    )
    LAST_RESULTS = res
    out = np.concatenate([res.results[c]["out"] for c in range(N_CORES)], axis=0)
    return out.reshape(N_FULL, 1, H, W)


def kernel(img: np.ndarray, kx: np.ndarray, ky: np.ndarray) -> np.ndarray:
    img = np.ascontiguousarray(np.asarray(img, dtype=np.float32))
    assert img.shape == (N_FULL, 3, H, W), img.shape

    dx_ = _rank1_decompose(kx) if os.environ.get("SOBEL_NO_SEP", "0") != "1" else None
    dy_ = _rank1_decompose(ky) if dx_ is not None else None
    if dx_ is not None and dy_ is not None:
        (axc, bx, gx_t), (ayc, by, gy_t) = dx_, dy_
        stat, stat_mini = _build_stationaries_sep(axc, bx, ayc, by)
        key = ("sep", tuple(np.round(gx_t, 12)), tuple(np.round(gy_t, 12)))
        if key not in _CACHE:
            _CACHE[key] = _build_program_sep(tuple(gx_t), tuple(gy_t))
        nc = _CACHE[key]
    else:
        stat, stat_mini = _build_stationaries(kx, ky)
        if "gen" not in _CACHE:
            _CACHE["gen"] = _build_program()
        nc = _CACHE["gen"]

    in_maps = [
        {
            "img": img[c * N_PER_CORE : (c + 1) * N_PER_CORE],
            "stat": stat,
            "stat_mini": stat_mini,
        }
        for c in range(N_CORES)
    ]
    return _run(nc, in_maps)
